# revision 9
# baseline (speedup 1.0000x reference)
"""Multi-head latent attention (MLA) Bass kernel for 8 TRN2 NeuronCores.

Sharding: tensor-parallel over heads x data-parallel over batch.
Core c (0..7) owns batch b = c//4 and head group g = c%4 (8 heads of 32).
Each core computes, for its batch:
    latent shard (S/4 tokens) -> AllGather across the 4-core batch group
    qT_h, kT_h (RoPE'd, transposed [head_dim, seq]) and v for its 8 heads
    attention with transposed scores [s_k, s_q]; the softmax normalizer is
    a DVE tree-sum of the exp tiles + 2 small ones-matmuls (not a full
    ones-matmul chain); no max-subtraction -- scores are O(1) by
    construction
    out-projection fused per query chunk: partial_out = attn @ Wo[rows]
Host sums the 4 partials per batch. One AllGather, no other collectives.

Compute dtype: bf16 on the TensorE inputs, fp32 PSUM accumulation.
"""

import sys

for _p in ("/opt/trn_rl_repo", "/root/.axon_site/_ro/trn_rl_repo"):
    if _p not in sys.path:
        sys.path.insert(0, _p)

import numpy as np
import ml_dtypes

import concourse.bacc as bacc
import concourse.mybir as mybir
import concourse.tile as tile
from concourse import bass_isa
from concourse.bass_utils import run_bass_kernel_spmd

BF = mybir.dt.bfloat16
F32 = mybir.dt.float32
BF_NP = ml_dtypes.bfloat16

# Full-problem constants (hardcoded per the self-contained-kernel contract).
D_MODEL = 4096
D_LATENT = 512
NUM_HEADS = 32
HEAD_DIM = 128
ROPE_THETA = 10000.0
BATCH, SEQ = 2, 2048
N_CORES = 8
HEADS_PER_CORE = NUM_HEADS // 4  # 4 head groups x 2 batches = 8 cores


def build_nc(S=SEQ, D=D_MODEL, L=D_LATENT, H=HEADS_PER_CORE, Dh=HEAD_DIM,
             NA=512, NC=512):
    """Build the single-core Bass program (SPMD across 8 cores)."""
    assert S % NA == 0 and S % 128 == 0 and D % 128 == 0 and L % 128 == 0
    KD = D // 128     # contraction chunks over d_model
    LD = L // 128     # contraction chunks over d_latent
    JA = S // NA      # seq chunks in projection phase
    JC = S // NC      # seq chunks in attention phase
    SK = S // 128     # key-position chunks
    HD1 = H * Dh      # this core's total head width (1024)
    ND = D // NC      # output-column chunks
    SQ = S // 4       # this core's latent shard width (batch group of 4)

    nc = bacc.Bacc("TRN2", target_bir_lowering=False)

    hsT_d = nc.declare_dram_parameter("hsT", [D, S], BF, isOutput=False)
    wq_d = nc.declare_dram_parameter("Wq", [D, HD1], BF, isOutput=False)
    wc_d = nc.declare_dram_parameter("Wc", [D, L], BF, isOutput=False)
    wk_d = nc.declare_dram_parameter("Wk", [L, HD1], BF, isOutput=False)
    wv_d = nc.declare_dram_parameter("Wv", [L, HD1], BF, isOutput=False)
    wo_d = nc.declare_dram_parameter("Wo", [HD1, D], BF, isOutput=False)
    cosq_d = nc.declare_dram_parameter("cosq", [Dh, S], BF, isOutput=False)
    sinq_d = nc.declare_dram_parameter("sinq", [Dh, S], BF, isOutput=False)
    cosk_d = nc.declare_dram_parameter("cosk", [Dh, S], BF, isOutput=False)
    sink_d = nc.declare_dram_parameter("sink", [Dh, S], BF, isOutput=False)
    out_d = nc.declare_dram_parameter("out", [S, D], F32, isOutput=True)
    hsl_d = nc.declare_dram_parameter("hsL", [D, SQ], BF, isOutput=False)
    latq_d = nc.dram_tensor("latq_dram", [L, SQ], BF)
    latg_d = nc.dram_tensor("latg_dram", [4 * L, SQ], BF)

    Exp = mybir.ActivationFunctionType.Exp
    half = Dh // 2

    with tile.TileContext(nc) as tc:
        with tc.tile_pool(name="consts", bufs=1) as const_pool:
            ones_sk = const_pool.tile([128, 128], BF)
            nc.vector.memset(ones_sk[:], 1.0)
            with tc.tile_pool(name="qT", bufs=1) as qT_pool:
                qT_t = [qT_pool.tile([Dh, S], BF, name=f"qT{h}") for h in range(H)]

                # ========== Phase A: latent shard + qT (with RoPE) ==========
                with tc.tile_pool(name="wqA", bufs=1) as wqA_pool, \
                     tc.tile_pool(name="wcA", bufs=1) as wcA_pool, \
                     tc.tile_pool(name="hsA", bufs=KD + 4) as hsA_pool, \
                     tc.tile_pool(name="ropeq", bufs=1) as ropeq_pool, \
                     tc.tile_pool(name="lq", bufs=4) as lq_pool, \
                     tc.tile_pool(name="tmpA", bufs=2) as tmpA_pool, \
                     tc.tile_pool(name="psA", bufs=4, space="PSUM") as psA_pool:

                    # wq in two column halves so q-proj h0-3 can start after
                    # half the weight bytes have landed
                    HW2 = HD1 // 2
                    wq_t = [[wqA_pool.tile([128, HW2], BF,
                                           name=f"wq{p}_{kd}")
                             for kd in range(KD)] for p in range(2)]
                    wc_t = [wcA_pool.tile([128, L], BF, name=f"wc{kd}")
                            for kd in range(KD)]
                    cosq_sb = ropeq_pool.tile([Dh, S], BF)
                    sinq_sb = ropeq_pool.tile([Dh, S], BF)

                    # --- latent shard (S/4 of seq) + AllGather across the
                    # 4-core batch group; hidden behind the qT loop below ---
                    # pairwise DMA issue so matmul kd unblocks as early as
                    # possible
                    hl_ch = []
                    for kd in range(KD):
                        nc.sync.dma_start(
                            out=wc_t[kd][:], in_=wc_d[kd * 128:(kd + 1) * 128, :])
                        t = hsA_pool.tile([128, SQ], BF, tag="hsA",
                                          name=f"hsL_{kd}")
                        nc.sync.dma_start(
                            out=t[:], in_=hsl_d[kd * 128:(kd + 1) * 128, :])
                        hl_ch.append(t)
                    for ld in range(LD):
                        ps = psA_pool.tile([128, SQ], F32, tag="psA",
                                           name=f"psLq{ld}")
                        for kd in range(KD):
                            nc.tensor.matmul(
                                ps[:],
                                wc_t[kd][:, ld * 128:(ld + 1) * 128],
                                hl_ch[kd][:],
                                start=(kd == 0), stop=(kd == KD - 1))
                        lq = lq_pool.tile([128, SQ], BF, tag="lq",
                                          name=f"lq{ld}")
                        nc.scalar.copy(lq[:], ps[:])
                        nc.sync.dma_start(
                            out=latq_d[ld * 128:(ld + 1) * 128, :], in_=lq[:])
                    nc.gpsimd.collective_compute(
                        "AllGather",
                        mybir.AluOpType.bypass,
                        replica_groups=[[0, 1, 2, 3], [4, 5, 6, 7]],
                        ins=[latq_d[:]],
                        outs=[latg_d[:]],
                    )

                    for j in range(JA):
                        jj = slice(j * NA, (j + 1) * NA)
                        hs_ch = []
                        for kd in range(KD):
                            if j == 0:
                                # wq_lo[kd] paired with hs[j0][kd] so the
                                # first head-group's chain unblocks earliest
                                nc.sync.dma_start(
                                    out=wq_t[0][kd][:],
                                    in_=wq_d[kd * 128:(kd + 1) * 128, 0:HW2])
                            t = hsA_pool.tile([128, NA], BF, tag="hsA",
                                              name=f"hsA_{j}_{kd}")
                            nc.sync.dma_start(
                                out=t[:], in_=hsT_d[kd * 128:(kd + 1) * 128, jj])
                            hs_ch.append(t)
                        if j == 0:
                            nc.sync.dma_start(out=cosq_sb[:], in_=cosq_d[:])
                            nc.sync.dma_start(out=sinq_sb[:], in_=sinq_d[:])
                            for kd in range(KD):
                                nc.sync.dma_start(
                                    out=wq_t[1][kd][:],
                                    in_=wq_d[kd * 128:(kd + 1) * 128, HW2:HD1])
                        for h in range(H):
                            hp, hq = divmod(h * Dh, HW2)
                            ps = psA_pool.tile([128, NA], F32, tag="psA",
                                               name=f"psQ{j}_{h}")
                            for kd in range(KD):
                                nc.tensor.matmul(
                                    ps[:], wq_t[hp][kd][:, hq:hq + Dh],
                                    hs_ch[kd][:],
                                    start=(kd == 0), stop=(kd == KD - 1))
                            t1 = tmpA_pool.tile([128, NA], F32, tag="t1",
                                                name=f"t1q{j}_{h}")
                            t2 = tmpA_pool.tile([128, NA], F32, tag="t2",
                                                name=f"t2q{j}_{h}")
                            nc.vector.tensor_mul(t1[:], ps[:], cosq_sb[:, jj])
                            nc.vector.tensor_mul(t2[0:half, :], ps[half:Dh, :],
                                                 sinq_sb[0:half, jj])
                            nc.vector.tensor_mul(t2[half:Dh, :], ps[0:half, :],
                                                 sinq_sb[half:Dh, jj])
                            nc.vector.tensor_add(qT_t[h][:, jj], t1[:], t2[:])

                # ========== Phase B: kT (with RoPE) + v; prefetch Wo ========
                with tc.tile_pool(name="kT", bufs=1) as kT_pool, \
                     tc.tile_pool(name="v", bufs=1) as v_pool, \
                     tc.tile_pool(name="wo", bufs=1) as wo_pool:
                    kT_t = [kT_pool.tile([Dh, S], BF, name=f"kT{h}")
                            for h in range(H)]
                    v_t = [v_pool.tile([128, HD1], BF, name=f"v{i}")
                           for i in range(SK)]
                    wo_t = [wo_pool.tile([128, D], BF, name=f"wo{h}")
                            for h in range(H)]

                    # One PSUM pool spans B and C' so C's score banks are
                    # disjoint from B's working banks (no WAR delay):
                    # tags: pb 2x1 bank (B kT/v psums + C' out-proj),
                    #       sc 2x2 banks (scores), pv 2x1 bank (pv + denom).
                    ps_cm = tc.tile_pool(name="pswork", bufs=2, space="PSUM")
                    ps_pool = ps_cm.__enter__()
                    with tc.tile_pool(name="wkv", bufs=1) as wkv_pool, \
                         tc.tile_pool(name="ropek", bufs=1) as ropek_pool, \
                         tc.tile_pool(name="latB", bufs=4 * LD) as latB_pool, \
                         tc.tile_pool(name="tmpB", bufs=1) as tmpB_pool:

                        wk_t = [wkv_pool.tile([128, HD1], BF, name=f"wk{ld}")
                                for ld in range(LD)]
                        wv_t = [wkv_pool.tile([128, HD1], BF, name=f"wv{ld}")
                                for ld in range(LD)]
                        # gathered latent lives row-major per source core r:
                        # rows r*L..(r+1)*L are seq cols r*SQ..(r+1)*SQ.
                        # All 16 chunks stay resident through B.
                        latj = [[None] * LD for _ in range(4)]
                        for r in range(4):
                            for ld in range(LD):
                                t = latB_pool.tile([128, SQ], BF, tag="latB",
                                                   name=f"latB_{r}_{ld}")
                                nc.sync.dma_start(
                                    out=t[:],
                                    in_=latg_d[r * L + ld * 128:
                                               r * L + (ld + 1) * 128, :])
                                latj[r][ld] = t
                        for ld in range(LD):
                            nc.sync.dma_start(
                                out=wk_t[ld][:],
                                in_=wk_d[ld * 128:(ld + 1) * 128, :])
                            nc.sync.dma_start(
                                out=wv_t[ld][:],
                                in_=wv_d[ld * 128:(ld + 1) * 128, :])
                        cosk_sb = ropek_pool.tile([Dh, S], BF)
                        sink_sb = ropek_pool.tile([Dh, S], BF)
                        nc.sync.dma_start(out=cosk_sb[:], in_=cosk_d[:])
                        nc.sync.dma_start(out=sink_sb[:], in_=sink_d[:])

                        # kT: h outer so each head's kT completes early and
                        # unblocks that head's score matmuls in C'.
                        NB = 512
                        for h in range(H):
                            for j in range(S // NB):
                                jj = slice(j * NB, (j + 1) * NB)
                                ps = ps_pool.tile([128, NB], F32, tag="pb",
                                                  name=f"psK{h}_{j}")
                                for ld in range(LD):
                                    nc.tensor.matmul(
                                        ps[:], wk_t[ld][:, h * Dh:(h + 1) * Dh],
                                        latj[j][ld][:],
                                        start=(ld == 0), stop=(ld == LD - 1))
                                t1 = tmpB_pool.tile([128, NB], F32, tag="t1b",
                                                    name=f"t1k{h}_{j}")
                                t2 = tmpB_pool.tile([128, NB], F32, tag="t2b",
                                                    name=f"t2k{h}_{j}")
                                nc.vector.tensor_mul(t1[:], ps[:], cosk_sb[:, jj])
                                nc.vector.tensor_mul(t2[0:half, :],
                                                     ps[half:Dh, :],
                                                     sink_sb[0:half, jj])
                                nc.vector.tensor_mul(t2[half:Dh, :],
                                                     ps[0:half, :],
                                                     sink_sb[half:Dh, jj])
                                nc.vector.tensor_add(kT_t[h][:, jj], t1[:], t2[:])

                        # Wo prefetch: after B's own loads in the DMA queues,
                        # well before the first out-projection needs it.
                        for h in range(H):
                            nc.sync.dma_start(
                                out=wo_t[h][:],
                                in_=wo_d[h * 128:(h + 1) * 128, :])

                        # v: lhsT = latent chunk [lat, seq128], rhs = wv
                        for i in range(SK):
                            r = (i * 128) // SQ
                            io = slice(i * 128 - r * SQ, (i + 1) * 128 - r * SQ)
                            for cch in range(HD1 // 512):
                                cc = slice(cch * 512, (cch + 1) * 512)
                                ps = ps_pool.tile([128, 512], F32, tag="pb",
                                                  name=f"psV{i}_{cch}")
                                for ld in range(LD):
                                    nc.tensor.matmul(
                                        ps[:], latj[r][ld][:, io],
                                        wv_t[ld][:, cc],
                                        start=(ld == 0), stop=(ld == LD - 1))
                                nc.scalar.copy(v_t[i][:, cc], ps[:])

                    # ===== Phase C': attention fused with out-projection =====
                    with tc.tile_pool(name="ET", bufs=8) as et_pool, \
                         tc.tile_pool(name="esum", bufs=2) as esum_pool, \
                         tc.tile_pool(name="rinv", bufs=2) as rinv_pool, \
                         tc.tile_pool(name="ats", bufs=H + 2) as ats_pool, \
                         tc.tile_pool(name="outst", bufs=3) as outst_pool:

                        assert SK % 2 == 0
                        for jc in range(JC):
                            jj = slice(jc * NC, (jc + 1) * NC)
                            ats_t = []
                            for h in range(H):
                                ets = []
                                for i2 in range(SK // 2):
                                    ps2 = ps_pool.tile(
                                        [128, 2 * NC], F32, tag="sc",
                                        name=f"sc{h}_{jc}_{i2}")
                                    for p in range(2):
                                        i = i2 * 2 + p
                                        nc.tensor.matmul(
                                            ps2[:, p * NC:(p + 1) * NC],
                                            kT_t[h][:, i * 128:(i + 1) * 128],
                                            qT_t[h][:, jj],
                                            start=True, stop=True)
                                    et = et_pool.tile([128, 2 * NC], BF,
                                                      tag="ET",
                                                      name=f"et{h}_{jc}_{i2}")
                                    nc.scalar.activation(et[:], ps2[:], Exp)
                                    ets.append(et)
                                # DVE tree-sum of the exp tiles for the
                                # softmax denominator (replaces a 16-matmul
                                # ones chain on PE).
                                es = esum_pool.tile([128, 2 * NC], BF,
                                                    tag="es", name=f"es{h}_{jc}")
                                nc.vector.tensor_add(es[:], ets[0][:], ets[1][:])
                                for i2 in range(2, SK // 2):
                                    nc.vector.tensor_add(es[:], es[:], ets[i2][:])
                                pv = ps_pool.tile([Dh, NC], F32, tag="pv",
                                                    name=f"pv{h}_{jc}")
                                for i2 in range(SK // 2):
                                    for p in range(2):
                                        i = i2 * 2 + p
                                        sl = ets[i2][:, p * NC:(p + 1) * NC]
                                        nc.tensor.matmul(
                                            pv[:],
                                            v_t[i][:, h * Dh:(h + 1) * Dh],
                                            sl, start=(i == 0),
                                            stop=(i == SK - 1))
                                rr = ps_pool.tile([128, NC], F32, tag="pv",
                                                    name=f"rr{h}_{jc}")
                                nc.tensor.matmul(rr[:], ones_sk[:],
                                                 es[:, 0:NC],
                                                 start=True, stop=False)
                                nc.tensor.matmul(rr[:], ones_sk[:],
                                                 es[:, NC:2 * NC],
                                                 start=False, stop=True)
                                rbs = rinv_pool.tile([128, NC], F32,
                                                     tag="rbs",
                                                     name=f"rbs{h}_{jc}")
                                nc.vector.reciprocal_approx_fast(
                                    rbs[:], rr[:])
                                ats = ats_pool.tile([Dh, NC], BF,
                                                    tag="ats",
                                                    name=f"ats{h}_{jc}")
                                nc.vector.tensor_mul(ats[:], pv[:], rbs[:])
                                ats_t.append(ats)

                            # out-projection for this query chunk
                            for tl in range(NC // 128):
                                toff = slice(tl * 128, (tl + 1) * 128)
                                tt = slice(jc * NC + tl * 128,
                                           jc * NC + (tl + 1) * 128)
                                for ncol in range(ND):
                                    cc = slice(ncol * NC, (ncol + 1) * NC)
                                    ps = ps_pool.tile([128, NC], F32, tag="pb",
                                                       name=f"psD{jc}_{tl}_{ncol}")
                                    for h in range(H):
                                        nc.tensor.matmul(
                                            ps[:], ats_t[h][:, toff],
                                            wo_t[h][:, cc],
                                            start=(h == 0), stop=(h == H - 1))
                                    st = outst_pool.tile([128, NC], F32,
                                                         tag="outst",
                                                         name=f"outst{jc}_{tl}_{ncol}")
                                    nc.scalar.copy(st[:], ps[:])
                                    nc.sync.dma_start(out=out_d[tt, cc],
                                                      in_=st[:])
                    ps_cm.__exit__(None, None, None)

    nc.compile()
    return nc


def host_inputs(hidden_states, Wq, Wc, Wk, Wv, Wo, S=SEQ, Dh=HEAD_DIM,
                heads_per_core=HEADS_PER_CORE, n_cores=N_CORES):
    """Shard + preprocess full fp32 inputs into per-core bf16 in_maps."""
    scale = 1.0 / np.sqrt(Dh)
    pos = np.arange(S, dtype=np.float32)
    inv_freq = 1.0 / (ROPE_THETA ** (np.arange(0, Dh, 2, dtype=np.float32) / Dh))
    freqs = pos[:, None] * inv_freq
    emb = np.concatenate([freqs, freqs], axis=-1)      # [S, Dh]
    cosT = np.cos(emb).T.copy()                        # [Dh, S]
    sinT = np.sin(emb).T.copy()
    sinT[: Dh // 2] *= -1.0                            # sign baked for the swap trick
    cosq = (cosT * scale).astype(BF_NP)
    sinq = (sinT * scale).astype(BF_NP)
    cosk = cosT.astype(BF_NP)
    sink = sinT.astype(BF_NP)

    hw = heads_per_core * Dh
    in_maps = []
    for c in range(n_cores):
        b, g = divmod(c, 4)
        cols = slice(g * hw, (g + 1) * hw)
        sq = S // 4
        in_maps.append({
            "hsT": np.ascontiguousarray(hidden_states[b].T).astype(BF_NP),
            "hsL": np.ascontiguousarray(
                hidden_states[b].T[:, g * sq:(g + 1) * sq]).astype(BF_NP),
            "Wq": np.ascontiguousarray(Wq[:, cols]).astype(BF_NP),
            "Wc": Wc.astype(BF_NP),
            "Wk": np.ascontiguousarray(Wk[:, cols]).astype(BF_NP),
            "Wv": np.ascontiguousarray(Wv[:, cols]).astype(BF_NP),
            "Wo": np.ascontiguousarray(Wo[cols, :]).astype(BF_NP),
            "cosq": cosq, "sinq": sinq, "cosk": cosk, "sink": sink,
        })
    return in_maps


_NC_CACHE = {}


def kernel(hidden_states, Wq, Wc, Wk, Wv, Wo):
    hidden_states = np.asarray(hidden_states, dtype=np.float32)
    if "nc" not in _NC_CACHE:
        _NC_CACHE["nc"] = build_nc()
    nc = _NC_CACHE["nc"]
    in_maps = host_inputs(hidden_states, np.asarray(Wq, np.float32),
                          np.asarray(Wc, np.float32), np.asarray(Wk, np.float32),
                          np.asarray(Wv, np.float32), np.asarray(Wo, np.float32))
    res = run_bass_kernel_spmd(nc, in_maps, list(range(N_CORES))).results
    B, S, D = BATCH, SEQ, D_MODEL
    out = np.zeros((B, S, D), dtype=np.float32)
    for c in range(N_CORES):
        out[c // 4] += res[c]["out"]
    return out
